# revision 24
# baseline (speedup 1.0000x reference)
"""Trainium2 Bass kernel for nn_GateCircuit (14-qubit batched gate circuit).

Math: the reference applies RX(x@W.T[:,i]) then RY(params[i]) on wire i of
|0...0> (a product state stays a product state since each gate hits a distinct
wire), then a CNOT ladder CNOT(i, i+1), then measures <Z_0>.  Qubit 0 is only
ever a CNOT *control*, so its marginal is untouched by the ladder; the
expectation collapses to the single-qubit value

    <Z_0> = cos(x @ W[0]) * cos(params[0])
    out   = sigmoid(<Z_0>)

Sharding: pure data parallel, batch 4096 split 512 per core across 8 cores.

Host prep (scalar/row transforms only; the 4MB matvec stays on-device):
  w' = W[0] / (2*pi) sent as bf16 hi + lo rows (exact to ~1.5e-5 rel, a
  1KB single-packet DMA keeps the ~107GB/s hardware DMA queues free for
  x), and a per-call least-squares fit of
  F(v) = sigmoid(c0*cos(2*pi*sqrt(v))) on v in [0, 0.25] -- smallest
  degree with dense-grid rel error < 5e-3 (4x under the 2e-2 gate) --
  baked into the NEFF as immediates; the compile cache is keyed on the
  coefficients, so a params change recompiles (correctness first).

On-device per core (DVE + one PE op -- no ACT tables, no table loads):
  wP  = ones[2,128]^T @ [whi;wlo] single bf16 K=2 PE matmul; PSUM fp32
                                  accumulation reconstructs w' exactly
  z'  = x @ w'                    4x DVE scalar_tensor_tensor + accumulator
                                  (z' = z/2pi; x laid out [128, 4, 256]:
                                  partition p holds rows 4p..4p+3; quarter
                                  DMAs balanced over the Scalar/Sync HW-DGE
                                  queues, dots ordered by landing time)
  k   = (z' + M) - M              M = 1.5*2^23: fp32 round-to-nearest
  v   = (k - z')^2                = y^2, |y| <= 0.5 (sign-free)
  out = F(v)                      Horner via TS/STT, immediate coeffs
"""

import math

import numpy as np

_NCORES = 8
_B = 4096
_F = 256
_BS = _B // _NCORES  # 512 samples per core
_NT = _BS // 128     # 4 samples per partition
_INV_TWO_PI = float(1.0 / (2.0 * math.pi))
_MAGIC = float(1.5 * 2 ** 23)  # fp32 round-to-nearest-integer bias
_CACHE: dict = {}


def _build(coeffs):
    import concourse.bacc as bacc
    import concourse.mybir as mybir
    import concourse.tile as tile

    f32 = mybir.dt.float32
    Alu = mybir.AluOpType

    nc = bacc.Bacc("TRN2", target_bir_lowering=False, debug=False,
                   num_devices=_NCORES)

    bf16 = mybir.dt.bfloat16
    x_d = nc.dram_tensor("x", [_BS, _F], f32, kind="ExternalInput")
    w_d = nc.dram_tensor("w", [2, _F], bf16, kind="ExternalInput")
    o_d = nc.dram_tensor("o", [_BS], f32, kind="ExternalOutput")

    with tile.TileContext(nc) as tc:
        with (
            tc.tile_pool(name="xin", bufs=1) as xpool,
            tc.tile_pool(name="scratch", bufs=2) as spool,
            tc.tile_pool(name="small", bufs=1) as zpool,
            tc.tile_pool(name="wps", bufs=1, space="PSUM") as ppool,
        ):
            # x quarter n holds rows 4p+n (1KB contiguous per partition).
            # Queues: Sync HW = [w row (single packet), q2, most of q3],
            # Scalar HW = [q0, q1, tail of q3] (~107GB/s each; balanced so
            # both finish together); dots run in landing order q0,q2,q1,q3.
            xt = xpool.tile([128, _NT * _F], f32)
            xr = x_d.ap().rearrange("(p n) f -> p (n f)", n=_NT)
            wrow = zpool.tile([2, _F], bf16)
            nc.sync.dma_start(wrow[:], w_d[:, :], single_packet=True)
            nc.scalar.dma_start(xt[:, 0:_F], xr[:, 0:_F])
            nc.sync.dma_start(xt[:, 2 * _F:3 * _F], xr[:, 2 * _F:3 * _F])
            nc.scalar.dma_start(xt[:, _F:2 * _F], xr[:, _F:2 * _F])
            sp = 3 * _F + 232
            nc.sync.dma_start(xt[:, 3 * _F:sp], xr[:, 3 * _F:sp])
            nc.scalar.dma_start(xt[:, sp:], xr[:, sp:])

            # broadcast w' to all partitions in one bf16 K=2 matmul:
            # wP[p, f] = sum_k ones[k, p] * wrow[k, f] = whi[f] + wlo[f]
            ones = zpool.tile([2, 128], bf16)
            nc.gpsimd.memset(ones[:], 1.0)
            wP = ppool.tile([128, _F], f32)
            nc.tensor.matmul(wP[:], ones[:], wrow[:], start=True, stop=True)

            # stage wP into SBUF while DVE idles (PSUM reads cost +62cyc/op)
            wS = zpool.tile([128, _F], f32)
            nc.vector.tensor_copy(wS[:], wP[:])

            # z'[p, n] = sum_f x[4p+n, f] * w'[f]
            z = zpool.tile([128, _NT], f32)
            for n in (0, 2, 1, 3):
                prod = spool.tile([128, _F], mybir.dt.bfloat16, name=f"prod{n}")
                nc.vector.scalar_tensor_tensor(
                    prod[:], xt[:, n * _F:(n + 1) * _F], 1.0, wS[:],
                    op0=Alu.mult, op1=Alu.mult,
                    accum_out=z[:, n:n + 1],
                )

            # v = (round(z') - z')^2 via the fp32 magic-number trick
            a1 = zpool.tile([128, _NT], f32)
            nc.vector.tensor_scalar(a1[:], z[:], 1.0, _MAGIC,
                                    op0=Alu.mult, op1=Alu.add)
            ny = zpool.tile([128, _NT], f32)
            nc.vector.scalar_tensor_tensor(ny[:], a1[:], -_MAGIC, z[:],
                                           op0=Alu.add, op1=Alu.subtract)
            v = zpool.tile([128, _NT], f32)
            nc.vector.tensor_tensor(v[:], ny[:], ny[:], op=Alu.mult)

            # out = f0 + v*(f1 + v*(f2 + ...)), immediates, any degree
            deg = len(coeffs) - 1
            h = zpool.tile([128, _NT], f32)
            nc.vector.tensor_scalar(h[:], v[:], float(coeffs[deg]),
                                    float(coeffs[deg - 1]),
                                    op0=Alu.mult, op1=Alu.add)
            g = zpool.tile([128, _NT], f32, name="g0")
            nc.vector.scalar_tensor_tensor(g[:], h[:], 0.0, v[:],
                                           op0=Alu.bypass, op1=Alu.mult)
            for i, k in enumerate(range(deg - 2, 0, -1)):
                g2 = zpool.tile([128, _NT], f32, name=f"g{i + 1}")
                nc.vector.scalar_tensor_tensor(g2[:], g[:], float(coeffs[k]),
                                               v[:], op0=Alu.add, op1=Alu.mult)
                g = g2
            ot = zpool.tile([128, _NT], f32)
            nc.vector.tensor_scalar(ot[:], g[:], 1.0, float(coeffs[0]),
                                    op0=Alu.mult, op1=Alu.add)

            nc.sync.dma_start(o_d.ap().rearrange("(p n) -> p n", n=_NT), ot[:])

    nc.compile()
    return nc


def _get_nc(coeffs):
    key = tuple(float(c) for c in coeffs)
    if _CACHE.get("key") != key:
        _CACHE["nc"] = _build(coeffs)
        _CACHE["key"] = key
    return _CACHE["nc"]


def _fit_coeffs(c0: float) -> np.ndarray:
    """Least-squares fit of sigmoid(c0*cos(2*pi*sqrt(v))) on v in [0,.25]
    on Chebyshev-spaced nodes (near-minimax).  Picks the smallest degree
    (>= 2) whose max rel error on a dense grid is < 5e-3 (4x under the
    2e-2 gate); each extra degree costs one ~165ns DVE op."""
    t = 0.5 * (1.0 - np.cos(np.pi * np.linspace(0.0, 1.0, 401))) * 0.25
    Ft = 1.0 / (1.0 + np.exp(-c0 * np.cos(2.0 * np.pi * np.sqrt(t))))
    vd = np.linspace(0.0, 0.25, 2001)
    Fd = 1.0 / (1.0 + np.exp(-c0 * np.cos(2.0 * np.pi * np.sqrt(vd))))
    for deg in range(2, 9):
        A = np.stack([t ** k for k in range(deg + 1)], axis=1)
        coef, *_ = np.linalg.lstsq(A, Ft, rcond=None)
        got = np.zeros_like(vd)
        for k in range(deg, -1, -1):
            got = got * vd + coef[k]
        rel = np.max(np.abs(got - Fd) / np.abs(Fd))
        if rel < 5e-3 or deg == 8:
            return coef.astype(np.float32)


def _in_maps(x, W):
    x = np.ascontiguousarray(np.asarray(x, dtype=np.float32))
    W = np.asarray(W, dtype=np.float32)
    import ml_dtypes
    w1 = (W[0] * _INV_TWO_PI).astype(np.float32)
    whi = w1.astype(ml_dtypes.bfloat16)
    wlo = (w1 - whi.astype(np.float32)).astype(ml_dtypes.bfloat16)
    wc = np.ascontiguousarray(np.stack([whi, wlo]))
    return [
        {"x": x[c * _BS:(c + 1) * _BS], "w": wc}
        for c in range(_NCORES)
    ]


def run_spmd(x, W, params, **kw):
    """Compile (cached per params) and run on 8 cores.

    Retries a few times: the axon-relayed device occasionally reports a
    transient NRT_EXEC_UNIT_UNRECOVERABLE that clears on the next attempt.
    """
    import time

    from concourse import bass_utils

    params = np.asarray(params, dtype=np.float32)
    coeffs = _fit_coeffs(math.cos(float(params[0])))
    nc = _get_nc(coeffs)
    in_maps = _in_maps(x, W)
    last = None
    for attempt in range(4):
        try:
            return bass_utils.run_bass_kernel_spmd(
                nc, in_maps, list(range(_NCORES)), **kw
            )
        except Exception as e:  # transient device/relay errors
            last = e
            time.sleep(2.0 * (attempt + 1))
    raise last


def kernel(x, W, params):
    res = run_spmd(x, W, params)
    return np.concatenate([res.results[c]["o"] for c in range(_NCORES)], axis=0)


# revision 26
# speedup vs baseline: 1.0527x; 1.0527x over previous
"""Trainium2 Bass kernel for nn_GateCircuit (14-qubit batched gate circuit).

Math: the reference applies RX(x@W.T[:,i]) then RY(params[i]) on wire i of
|0...0> (a product state stays a product state since each gate hits a distinct
wire), then a CNOT ladder CNOT(i, i+1), then measures <Z_0>.  Qubit 0 is only
ever a CNOT *control*, so its marginal is untouched by the ladder; the
expectation collapses to the single-qubit value

    <Z_0> = cos(x @ W[0]) * cos(params[0])
    out   = sigmoid(<Z_0>)

Sharding: pure data parallel, batch 4096 split 512 per core across 8 cores.

Host prep (scalar/row transforms only; the 4MB matvec stays on-device):
  w' = W[0] / (2*pi) sent as bf16 hi + lo rows (exact to ~1.5e-5 rel, a
  1KB single-packet DMA keeps the ~107GB/s hardware DMA queues free for
  x), and a per-call least-squares fit of
  F(v) = sigmoid(c0*cos(2*pi*sqrt(v))) on v in [0, 0.25] -- smallest
  degree with dense-grid rel error < 5e-3 (4x under the 2e-2 gate) --
  baked into the NEFF as immediates; the compile cache is keyed on the
  coefficients, so a params change recompiles (correctness first).

On-device per core (DVE + one PE op -- no ACT tables, no table loads):
  wP  = ones[2,128]^T @ [whi;wlo] single bf16 K=2 PE matmul; PSUM fp32
                                  accumulation reconstructs w' exactly
  z'  = x @ w'                    4x DVE scalar_tensor_tensor + accumulator
                                  (z' = z/2pi; x laid out [128, 4, 256]:
                                  partition p holds rows 4p..4p+3; quarter
                                  DMAs balanced over the Scalar/Sync HW-DGE
                                  queues, dots ordered by landing time)
  k   = (z' + M) - M              M = 1.5*2^23: fp32 round-to-nearest
  v   = (k - z')^2                = y^2, |y| <= 0.5 (sign-free)
  out = F(v)                      Horner via TS/STT, immediate coeffs
"""

import math

import numpy as np

_NCORES = 8
_B = 4096
_F = 256
_BS = _B // _NCORES  # 512 samples per core
_NT = _BS // 128     # 4 samples per partition
_INV_TWO_PI = float(1.0 / (2.0 * math.pi))
_MAGIC = float(1.5 * 2 ** 23)  # fp32 round-to-nearest-integer bias
_CACHE: dict = {}


def _build(coeffs, use_f16):
    import concourse.bacc as bacc
    import concourse.mybir as mybir
    import concourse.tile as tile

    f32 = mybir.dt.float32
    Alu = mybir.AluOpType

    nc = bacc.Bacc("TRN2", target_bir_lowering=False, debug=False,
                   num_devices=_NCORES)

    bf16 = mybir.dt.bfloat16
    xdt = mybir.dt.float16 if use_f16 else f32
    x_d = nc.dram_tensor("x", [_BS, _F], xdt, kind="ExternalInput")
    w_d = nc.dram_tensor("w", [2, _F], bf16, kind="ExternalInput")
    o_d = nc.dram_tensor("o", [_BS], f32, kind="ExternalOutput")

    with tile.TileContext(nc) as tc:
        with (
            tc.tile_pool(name="xin", bufs=1) as xpool,
            tc.tile_pool(name="scratch", bufs=2) as spool,
            tc.tile_pool(name="small", bufs=1) as zpool,
            tc.tile_pool(name="wps", bufs=1, space="PSUM") as ppool,
        ):
            # x quarter n holds rows 4p+n (1KB contiguous per partition).
            # Queues: Sync HW = [w row (single packet), q2, most of q3],
            # Scalar HW = [q0, q1, tail of q3] (~107GB/s each; balanced so
            # both finish together); dots run in landing order q0,q2,q1,q3.
            xt = xpool.tile([128, _NT * _F], xdt)
            xr = x_d.ap().rearrange("(p n) f -> p (n f)", n=_NT)
            wrow = zpool.tile([2, _F], bf16)
            nc.sync.dma_start(wrow[:], w_d[:, :], single_packet=True)
            nc.scalar.dma_start(xt[:, 0:_F], xr[:, 0:_F])
            nc.sync.dma_start(xt[:, 2 * _F:3 * _F], xr[:, 2 * _F:3 * _F])
            nc.scalar.dma_start(xt[:, _F:2 * _F], xr[:, _F:2 * _F])
            nc.sync.dma_start(xt[:, 3 * _F:], xr[:, 3 * _F:])

            # broadcast w' to all partitions in one bf16 K=2 matmul:
            # wP[p, f] = sum_k ones[k, p] * wrow[k, f] = whi[f] + wlo[f]
            ones = zpool.tile([2, 128], bf16)
            nc.gpsimd.memset(ones[:], 1.0)
            wP = ppool.tile([128, _F], f32)
            nc.tensor.matmul(wP[:], ones[:], wrow[:], start=True, stop=True)

            # stage wP into SBUF while DVE idles (PSUM reads cost +62cyc/op);
            # in the f16 path the copy also downcasts w to f16 for 2x STT rate
            wS = zpool.tile([128, _F], xdt)
            nc.vector.tensor_copy(wS[:], wP[:])

            # z'[p, n] = sum_f x[4p+n, f] * w'[f]
            z = zpool.tile([128, _NT], f32)
            for n in (0, 2, 1, 3):
                prod = spool.tile([128, _F], bf16 if not use_f16 else mybir.dt.float16,
                                  name=f"prod{n}")
                nc.vector.scalar_tensor_tensor(
                    prod[:], xt[:, n * _F:(n + 1) * _F], 1.0, wS[:],
                    op0=Alu.mult, op1=Alu.mult,
                    accum_out=z[:, n:n + 1],
                )

            # v = (round(z') - z')^2 via the fp32 magic-number trick
            a1 = zpool.tile([128, _NT], f32)
            nc.vector.tensor_scalar(a1[:], z[:], 1.0, _MAGIC,
                                    op0=Alu.mult, op1=Alu.add)
            ny = zpool.tile([128, _NT], f32)
            nc.vector.scalar_tensor_tensor(ny[:], a1[:], -_MAGIC, z[:],
                                           op0=Alu.add, op1=Alu.subtract)
            v = zpool.tile([128, _NT], f32)
            nc.vector.tensor_tensor(v[:], ny[:], ny[:], op=Alu.mult)

            # out = f0 + v*(f1 + v*(f2 + ...)), immediates, any degree
            deg = len(coeffs) - 1
            h = zpool.tile([128, _NT], f32)
            nc.vector.tensor_scalar(h[:], v[:], float(coeffs[deg]),
                                    float(coeffs[deg - 1]),
                                    op0=Alu.mult, op1=Alu.add)
            g = zpool.tile([128, _NT], f32, name="g0")
            nc.vector.scalar_tensor_tensor(g[:], h[:], 0.0, v[:],
                                           op0=Alu.bypass, op1=Alu.mult)
            for i, k in enumerate(range(deg - 2, 0, -1)):
                g2 = zpool.tile([128, _NT], f32, name=f"g{i + 1}")
                nc.vector.scalar_tensor_tensor(g2[:], g[:], float(coeffs[k]),
                                               v[:], op0=Alu.add, op1=Alu.mult)
                g = g2
            ot = zpool.tile([128, _NT], f32)
            nc.vector.tensor_scalar(ot[:], g[:], 1.0, float(coeffs[0]),
                                    op0=Alu.mult, op1=Alu.add)

            nc.sync.dma_start(o_d.ap().rearrange("(p n) -> p n", n=_NT), ot[:])

    nc.compile()
    return nc


def _get_nc(coeffs, use_f16):
    key = (use_f16, tuple(float(c) for c in coeffs))
    if _CACHE.get("key") != key:
        _CACHE["nc"] = _build(coeffs, use_f16)
        _CACHE["key"] = key
    return _CACHE["nc"]


def _fit_coeffs(c0: float) -> np.ndarray:
    """Least-squares fit of sigmoid(c0*cos(2*pi*sqrt(v))) on v in [0,.25]
    on Chebyshev-spaced nodes (near-minimax).  Picks the smallest degree
    (>= 2) whose max rel error on a dense grid is < 5e-3 (4x under the
    2e-2 gate); each extra degree costs one ~165ns DVE op."""
    t = 0.5 * (1.0 - np.cos(np.pi * np.linspace(0.0, 1.0, 401))) * 0.25
    Ft = 1.0 / (1.0 + np.exp(-c0 * np.cos(2.0 * np.pi * np.sqrt(t))))
    vd = np.linspace(0.0, 0.25, 2001)
    Fd = 1.0 / (1.0 + np.exp(-c0 * np.cos(2.0 * np.pi * np.sqrt(vd))))
    for deg in range(2, 9):
        A = np.stack([t ** k for k in range(deg + 1)], axis=1)
        coef, *_ = np.linalg.lstsq(A, Ft, rcond=None)
        got = np.zeros_like(vd)
        for k in range(deg, -1, -1):
            got = got * vd + coef[k]
        rel = np.max(np.abs(got - Fd) / np.abs(Fd))
        if rel < 5e-3 or deg == 8:
            return coef.astype(np.float32), float(rel)


def _in_maps(x, W, use_f16):
    import ml_dtypes
    xdt = np.float16 if use_f16 else np.float32
    x = np.ascontiguousarray(np.asarray(x).astype(xdt))
    W = np.asarray(W, dtype=np.float32)
    w1 = (W[0] * _INV_TWO_PI).astype(np.float32)
    whi = w1.astype(ml_dtypes.bfloat16)
    wlo = (w1 - whi.astype(np.float32)).astype(ml_dtypes.bfloat16)
    wc = np.ascontiguousarray(np.stack([whi, wlo]))
    return [
        {"x": x[c * _BS:(c + 1) * _BS], "w": wc}
        for c in range(_NCORES)
    ]


def run_spmd(x, W, params, **kw):
    """Compile (cached per params) and run on 8 cores.

    Retries a few times: the axon-relayed device occasionally reports a
    transient NRT_EXEC_UNIT_UNRECOVERABLE that clears on the next attempt.
    """
    import time

    from concourse import bass_utils

    params = np.asarray(params, dtype=np.float32)
    c0 = math.cos(float(params[0]))
    coeffs, poly_rel = _fit_coeffs(c0)
    # f16 x/w adds <= ~1.2e-2*|c0| rel error on top of the poly fit; use it
    # only when the combined bound keeps >40% headroom under the 2e-2 gate
    use_f16 = (poly_rel + 0.012 * abs(c0)) < 1.2e-2
    nc = _get_nc(coeffs, use_f16)
    in_maps = _in_maps(x, W, use_f16)
    last = None
    for attempt in range(4):
        try:
            return bass_utils.run_bass_kernel_spmd(
                nc, in_maps, list(range(_NCORES)), **kw
            )
        except Exception as e:  # transient device/relay errors
            last = e
            time.sleep(2.0 * (attempt + 1))
    raise last


def kernel(x, W, params):
    res = run_spmd(x, W, params)
    return np.concatenate([res.results[c]["o"] for c in range(_NCORES)], axis=0)
